# revision 19
# baseline (speedup 1.0000x reference)
"""Tensor-parallel Llama layer on 8 Trainium2 NeuronCores (Bass/Tile).

Sharding: TP per the hint — wq/wk/wv/wg/wh column-sharded (4 q-heads + 1 kv
head + 1792 ffn rows per core), wo/wf row-sharded with ReduceScatter after
attention-out and ffn-out; sequence-parallel RMSNorms (256 tokens/core) with
AllGather of the normed activations (bf16).

Activations are kept feature-major (x.T layout) on chip so every projection
is a plain lhsT.T @ rhs with contraction on the partition axis. Weights are
pre-transposed and pre-cast to bf16 on the host (host prep is free).

v2 perf notes (from the v1 trace):
- PE runs GPIO-throttled at 13/16 of 2.4GHz; a dense N=512 matmul stream
  issues at ~263ns. LDWEIGHTS is already hidden; the wins are all in
  eliminating PE idle, not in restructuring matmuls.
- Collectives execute on ONE serial CC stream with a ~40us per-op floor:
  use few, large chunks (2 per collective), fire them as early as possible,
  and issue them from a dedicated queue (sync) since collective_compute
  blocks its issuing queue until completion.
- The PE MATMUL queue is strict FIFO: attention's score->exp->AV chain
  head-of-line blocks. Issue scores in waves of 3 so exp (scalar) pipelines
  behind the matmul stream.
- Prefetch loads on sync, collective-gated loads + partial-sum stores on
  scalar, collectives + residual/output stores on gpsimd.
"""
import sys

sys.path.insert(0, '/opt/trn_rl_repo')
from contextlib import ExitStack

import numpy as np
import ml_dtypes

import concourse.bass as bass
import concourse.tile as tile
from concourse import bacc, mybir
from concourse.bass_utils import run_bass_kernel_spmd

AF = mybir.ActivationFunctionType
ALU = mybir.AluOpType
BF16 = mybir.dt.bfloat16
F32 = mybir.dt.float32
F8E4 = mybir.dt.float8e4

CORES = 8
DH = 128
EPS = 1e-5
TBLK = 512
NEG_BIG = -1e30

FULL_CFG = dict(N=2048, D=4096, QH=4, FC=1792)

# CoreSim doesn't implement Silu; set True to build with Sigmoid + an extra
# multiply (same math) for simulator validation.
SILU_VIA_SIGMOID = False

# ReduceScatter the attention/ffn partial sums in bf16 (halves collective
# time); flip to False if accuracy needs the headroom.
RS_BF16 = True


def build_module(cfg):
    N, D, QH, FC = cfg['N'], cfg['D'], cfg['QH'], cfg['FC']
    C = CORES
    NB = N // C            # tokens per core block
    TT = NB // 128         # token tiles per core block
    KP = D // 128          # d_model contraction chunks
    NBLK = N // TBLK       # matmul token blocks
    BPT = TBLK // NB       # DRAM token-blocks per matmul token block
    KCH = N // DH          # attention k chunks
    QT = N // TBLK         # q tiles per head
    FM = FC // DH          # ffn M tiles per core
    T2 = N // 2            # ffn token half
    NS2 = T2 // TBLK       # 512-subblocks per ffn half
    BPH = C // 2           # DRAM token-blocks per ffn half
    MQKV = QH + 2
    scale = float(1.0 / np.sqrt(DH))

    nc = bacc.Bacc("TRN2", target_bir_lowering=False, debug=False, num_devices=C)

    x_c = nc.dram_tensor("x_c", [NB, D], F32, kind="ExternalInput")
    wqT = nc.dram_tensor("wqT", [D, QH * DH], BF16, kind="ExternalInput")
    wkT = nc.dram_tensor("wkT", [D, DH], BF16, kind="ExternalInput")
    wvT = nc.dram_tensor("wvT", [D, DH], BF16, kind="ExternalInput")
    woT = nc.dram_tensor("woT", [D, D], BF16, kind="ExternalInput")
    wgT = nc.dram_tensor("wgT", [D, FC], BF16, kind="ExternalInput")
    whT = nc.dram_tensor("whT", [D, FC], BF16, kind="ExternalInput")
    wfT = nc.dram_tensor("wfT", [FC, D], BF16, kind="ExternalInput")
    rcosT = nc.dram_tensor("rcosT", [DH, N], F32, kind="ExternalInput")
    rsinT = nc.dram_tensor("rsinT", [DH, N], F32, kind="ExternalInput")
    swapT = nc.dram_tensor("swapT", [DH, DH], BF16, kind="ExternalInput")
    diagneg = nc.dram_tensor("diagneg", [DH, DH], BF16, kind="ExternalInput")
    identb = nc.dram_tensor("identb", [128, 128], BF16, kind="ExternalInput")
    onesc = nc.dram_tensor("onesc", [128, 128], BF16, kind="ExternalInput")
    masks = nc.dram_tensor("masks", [4, 128, TBLK], BF16, kind="ExternalInput")
    out_c = nc.dram_tensor("out_c", [NB, D], F32, kind="ExternalOutput")

    RSDT = BF16 if RS_BF16 else F32
    # Partial-sum exchanges are chunked ReduceScatters, each fired as soon
    # as its producer features are done (RS cost is ~linear in bytes, ~12us/MB
    # on this fabric; A2A and fp8 variants measured worse/too-inaccurate).
    AG_CUTS = [0, 32]                # norm-AllGathers (single shot: the
                                     # consumers need all features anyway)
    RS2_CUTS = [0, 12, 24, 28, 32]   # ffn-out ReduceScatter (small tail)

    def ch_of(cuts, kp):
        for c in range(len(cuts) - 1):
            if kp < cuts[c + 1]:
                return c, kp - cuts[c]
        raise ValueError

    with tile.TileContext(nc) as tc, ExitStack() as top:
        dram = top.enter_context(tc.tile_pool(name="dram", bufs=1, space="DRAM"))

        def dram_chunks(nm, cuts, mul, dt, shared=False):
            kw = dict(addr_space="Shared") if shared else {}
            return [dram.tile([(cuts[i + 1] - cuts[i]) * 128 * mul, NB], dt,
                              tag=f"{nm}{i}", name=f"{nm}{i}", **kw)
                    for i in range(len(cuts) - 1)]

        r2d = dram.tile([NB, D], F32, tag="r2d", name="r2d")
        hT_in_ch = dram_chunks("hT_in", AG_CUTS, 1, BF16)
        hT_all_ch = dram_chunks("hT_all", AG_CUTS, C, BF16, shared=True)
        # attention-out exchange: AllToAll re-shards a from head-sharded to
        # token-sharded (2.1MB per core vs a 16.8MB ReduceScatter); wo then
        # runs locally per core over its own tokens with full Wo streamed.
        a2a_in = [dram.tile([C * 2 * DH, NB], BF16, tag=f"a2ai{p}",
                            name=f"a2ai{p}") for p in range(2)]
        a2a_out = [dram.tile([C * 2 * DH, NB], BF16, tag=f"a2ao{p}",
                             name=f"a2ao{p}") for p in range(2)]
        h2T_in_ch = dram_chunks("h2T_in", AG_CUTS, 1, BF16)
        h2T_all_ch = dram_chunks("h2T_all", AG_CUTS, C, BF16, shared=True)
        fpart_ch = dram_chunks("fpart", RS2_CUTS, C, RSDT)
        fred_ch = dram_chunks("fred", RS2_CUTS, 1, RSDT)

        # ---- constants resident in SBUF (loads on gpsimd queue) ----
        const = top.enter_context(tc.tile_pool(name="const", bufs=1))
        identb_sb = const.tile([128, 128], BF16, tag="identb", name="identb")
        swap_sb = const.tile([DH, DH], BF16, tag="swap", name="swap")
        diag_sb = const.tile([DH, DH], BF16, tag="diag", name="diag")
        ones_sb = const.tile([128, 128], BF16, tag="ones", name="ones")
        masks_sb = const.tile([128, 4 * TBLK], BF16, tag="masks", name="masks")

        # ---- shared PSUM pools (4+3+1 = 8 banks) ----
        ps_acc = top.enter_context(tc.tile_pool(name="ps_acc", bufs=4, space="PSUM"))
        ps_tmp = top.enter_context(tc.tile_pool(name="ps_tmp", bufs=3, space="PSUM"))
        ps_sml = top.enter_context(tc.tile_pool(name="ps_sml", bufs=1, space="PSUM"))

        # ---- attention residents (freed after P3; opened last for LIFO) ----
        attn_ctx = ExitStack()
        attn = attn_ctx.enter_context(tc.tile_pool(name="attn", bufs=1))
        rcos_sb = attn.tile([DH, N], F32, tag="rcos", name="rcos")
        rsin_sb = attn.tile([DH, N], F32, tag="rsin", name="rsin")
        qrot = [attn.tile([DH, N], BF16, tag=f"qrot{h}", name=f"qrot{h}") for h in range(QH)]
        krot = attn.tile([DH, N], BF16, tag="krot", name="krot")
        vsb = attn.tile([DH, N], BF16, tag="vsb", name="vsb")
        vtok = attn.tile([128, KCH * DH], BF16, tag="vtok", name="vtok")
        aT = [attn.tile([DH, N], BF16, tag=f"aT{h}", name=f"aT{h}") for h in range(QH)]

        def seqpar_norm_and_gather(src_tiles, dst_chunks, out_chunks, pool,
                                   pspool, prefix, sq_partials=None):
            """src_tiles: TT SBUF tiles [128, D] f32 (token-major rows of this
            core's block). Phase A: RMS-normalize each row (scalar+vector).
            Phase B: chunk-ordered transpose to feature-major, store, and fire
            the AllGather for each chunk as soon as its stores are queued."""
            htoks = []
            for t in range(TT):
                xt = src_tiles[t]
                if sq_partials is None:
                    sq = pool.tile([128, D], F32, tag=f"{prefix}sq", name=f"{prefix}sq")
                    ssum = pool.tile([128, 1], F32, tag=f"{prefix}ss", name=f"{prefix}ss")
                    nc.scalar.activation(sq[:], xt[:], AF.Square, accum_out=ssum[:])
                else:
                    acc = sq_partials[t][0]
                    for pi, p in enumerate(sq_partials[t][1:]):
                        ns = pool.tile([128, 1], F32, tag=f"{prefix}ss{pi % 2}",
                                       name=f"{prefix}ss")
                        nc.vector.tensor_tensor(ns[:], acc[:], p[:], op=ALU.add)
                        acc = ns
                    ssum = acc
                var = pool.tile([128, 1], F32, tag=f"{prefix}var", name=f"{prefix}var")
                nc.vector.tensor_scalar(
                    out=var[:], in0=ssum[:], scalar1=1.0 / D, scalar2=EPS,
                    op0=ALU.mult, op1=ALU.add)
                sv = pool.tile([128, 1], F32, tag=f"{prefix}sv", name=f"{prefix}sv")
                nc.scalar.activation(sv[:], var[:], AF.Sqrt)
                rstd = pool.tile([128, 1], F32, tag=f"{prefix}rstd", name=f"{prefix}rstd")
                nc.vector.reciprocal(rstd[:], sv[:])
                htok = pool.tile([128, D], BF16, tag=f"{prefix}h{t}", name=f"{prefix}h{t}")
                nc.vector.tensor_scalar_mul(htok[:], xt[:], rstd[:])
                htoks.append(htok)
            for ch in range(len(AG_CUTS) - 1):
                for t in range(TT):
                    for gl in range((AG_CUTS[ch + 1] - AG_CUTS[ch]) // 4):
                        g = AG_CUTS[ch] // 4 + gl
                        ps = pspool.tile([128, 512], BF16, tag="tmp", name="tps")
                        for q4 in range(4):
                            dd = 4 * g + q4
                            nc.tensor.transpose(
                                ps[:, 128 * q4:128 * (q4 + 1)],
                                htoks[t][:, 128 * dd:128 * (dd + 1)], identb_sb[:])
                        ev = pool.tile([128, 512], BF16, tag=f"{prefix}ev{gl % 4}", name=f"{prefix}ev")
                        if g % 2 == 0:
                            nc.vector.tensor_copy(ev[:], ps[:])
                        else:
                            nc.scalar.activation(ev[:], ps[:], AF.Copy)
                        r0 = 128 * 4 * gl
                        nc.gpsimd.dma_start(
                            dst_chunks[ch][r0:r0 + 512, 128 * t:128 * (t + 1)]
                            .rearrange("(q d) t -> d q t", q=4),
                            ev[:].rearrange("p (q t) -> p q t", q=4))
                nc.gpsimd.collective_compute(
                    "AllGather", ALU.bypass, replica_groups=[list(range(C))],
                    ins=[dst_chunks[ch][:].opt()], outs=[out_chunks[ch][:].opt()])

        # ================= P0: norm1 (seq-parallel) + chunked AllGather ====
        p0_ctx = ExitStack()
        p0 = p0_ctx.enter_context(tc.tile_pool(name="p0", bufs=1))
        x_tiles = []
        for t in range(TT):
            xt = p0.tile([128, D], F32, tag=f"x{t}", name=f"x{t}")
            nc.sync.dma_start(xt[:], x_c.ap()[128 * t:128 * (t + 1), :])
            x_tiles.append(xt)
        # consts after x on the same load queue
        nc.sync.dma_start(identb_sb[:], identb.ap())
        nc.sync.dma_start(swap_sb[:], swapT.ap())
        nc.sync.dma_start(diag_sb[:], diagneg.ap())
        nc.sync.dma_start(ones_sb[:], onesc.ap())
        nc.sync.dma_start(
            masks_sb[:].rearrange("p (r t) -> p r t", r=4),
            masks.ap().rearrange("r p t -> p r t"))
        seqpar_norm_and_gather(x_tiles, hT_in_ch, hT_all_ch, p0, ps_tmp, "n1")
        p0_ctx.close()

        hT_views = [hT_all_ch[ch][:].rearrange("(b d) t -> d b t", b=C)
                    for ch in range(len(AG_CUTS) - 1)]

        # ================= P1: QKV + RoPE (per token block) =================
        with ExitStack() as ctx:
            wsl = ctx.enter_context(tc.tile_pool(name="qkv_w", bufs=1))
            rhsp = ctx.enter_context(tc.tile_pool(name="qkv_rhs", bufs=2))
            ep = ctx.enter_context(tc.tile_pool(name="qkv_ep", bufs=3))
            # QKV weight slabs are small (6 x 8KB/partition bf16): load once
            slabs = []
            for m in range(MQKV):
                slab = wsl.tile([128, KP * 128], BF16, tag=f"w{m}", name=f"w{m}")
                if m < QH:
                    src = wqT.ap()[:, 128 * m:128 * (m + 1)]
                elif m == QH:
                    src = wkT.ap()
                else:
                    src = wvT.ap()
                nc.sync.dma_start(
                    slab[:].rearrange("p (k m) -> p k m", m=128),
                    src.rearrange("(k p) m -> p k m", p=128))
                slabs.append(slab)
            # rope tables after slabs (needed later than slab m=0)
            nc.sync.dma_start(rcos_sb[:], rcosT.ap())
            nc.sync.dma_start(rsin_sb[:], rsinT.ap())

            def rope(dst, src_sb, ps_swap, sl):
                """dst[:, sl] = src*cos + (P@src)*sin; src_sb bf16, ps_swap psum."""
                t1 = ep.tile([128, TBLK], F32, tag="rope_t1", name="rope_t1")
                nc.vector.tensor_tensor(t1[:], src_sb[:], rcos_sb[:, sl], op=ALU.mult)
                t2 = ep.tile([128, TBLK], F32, tag="rope_t2", name="rope_t2")
                nc.vector.tensor_tensor(t2[:], ps_swap[:], rsin_sb[:, sl], op=ALU.mult)
                nc.vector.tensor_tensor(dst[:, sl], t1[:], t2[:], op=ALU.add)

            for nb in range(NBLK):
                sl = slice(TBLK * nb, TBLK * (nb + 1))
                # one rhs load per (nb, kp), shared by both M-groups
                rtiles = []
                for kp in range(KP):
                    rt = rhsp.tile([128, TBLK], BF16, tag=f"rhs{kp}", name=f"rhs{kp}")
                    chq, kpl = ch_of(AG_CUTS, kp)
                    nc.sync.dma_start(
                        rt[:].rearrange("p (b t) -> p b t", b=BPT),
                        hT_views[chq][128 * kpl:128 * (kpl + 1),
                                      BPT * nb:BPT * (nb + 1), :])
                    rtiles.append(rt)
                # emit both m-groups' matmul chains first (dense PE stream),
                # then the rope/evac work, so the PE queue never head-of-line
                # blocks on scalar evacs.
                pending = []
                for hm in range(2):
                    group = list(range(3 * hm, min(3 * (hm + 1), MQKV)))
                    gacc = {m: ps_acc.tile([128, TBLK], F32, tag="acc", name="acc") for m in group}
                    for kp in range(KP):
                        for m in group:
                            nc.tensor.matmul(
                                gacc[m][:], slabs[m][:, 128 * kp:128 * (kp + 1)],
                                rtiles[kp][:], start=(kp == 0), stop=(kp == KP - 1))
                    # evacs (scalar) can run while the next group's chains run
                    for m in group:
                        ps = gacc[m]
                        if m <= QH:  # q heads and k need rope
                            sb = ep.tile([128, TBLK], BF16, tag=f"qk_sb{m}", name="qk_sb")
                            nc.scalar.activation(sb[:], ps[:], AF.Copy)
                            pending.append((m, sb))
                        else:  # v: plain copy
                            nc.scalar.activation(vsb[:, sl], ps[:], AF.Copy)
                # rope swaps (PE) + vector rope, after all chains queued
                for m, sb in pending:
                    ps_swap = ps_tmp.tile([128, TBLK], F32, tag="tmp", name="swp")
                    nc.tensor.matmul(ps_swap[:], swap_sb[:], sb[:],
                                     start=True, stop=True)
                    dst = qrot[m] if m < QH else krot
                    rope(dst, sb, ps_swap, sl)
                # transpose this block's v chunks to token-major
                for q4 in range(BPT * NB // 128):
                    i = (TBLK * nb) // 128 + q4
                    psv = ps_tmp.tile([128, 512], BF16, tag="tmp", name="vtp")
                    nc.tensor.transpose(
                        psv[:, 128 * (i % 4):128 * (i % 4) + 128],
                        vsb[:, 128 * i:128 * (i + 1)], identb_sb[:])
                    nc.vector.tensor_copy(
                        vtok[:, 128 * i:128 * (i + 1)],
                        psv[:, 128 * (i % 4):128 * (i % 4) + 128])

        # ================= P2: attention =================
        # Per (head, q-tile): k-chunks processed in waves of 3 so the PE
        # matmul stream (score/AV/lsum) never head-of-line blocks on the
        # scalar exp.
        with ExitStack() as ctx:
            pp = ctx.enter_context(tc.tile_pool(name="att_p", bufs=6))
            ap2 = ctx.enter_context(tc.tile_pool(name="att_t", bufs=4))
            for h in range(QH):
                for j in range(QT):
                    qsl = slice(TBLK * j, TBLK * (j + 1))
                    nk = (TBLK * (j + 1)) // DH
                    ps_a = ps_acc.tile([128, TBLK], F32, tag="acc", name="acc")
                    ps_l = ps_sml.tile([1, TBLK], F32, tag="lsum", name="lsum")
                    kpj = TBLK // DH  # k chunks per q tile (straddle count)
                    for i0 in range(0, nk, 3):
                        wave = range(i0, min(i0 + 3, nk))
                        pts = {}
                        for i in wave:
                            ps_s = ps_tmp.tile([128, TBLK], F32, tag="tmp", name="score")
                            diagonal = i >= kpj * j
                            nc.tensor.matmul(
                                ps_s[:], krot[:, DH * i:DH * (i + 1)], qrot[h][:, qsl],
                                start=True, stop=not diagonal)
                            if diagonal:
                                ri = i - kpj * j
                                nc.tensor.matmul(
                                    ps_s[:], diag_sb[:],
                                    masks_sb[:, TBLK * ri:TBLK * (ri + 1)],
                                    start=False, stop=True)
                            pt = pp.tile([128, TBLK], BF16, tag="p", name="p")
                            nc.scalar.activation(pt[:], ps_s[:], AF.Exp, scale=scale)
                            pts[i] = pt
                        for i in wave:
                            nc.tensor.matmul(ps_a[:], vtok[:, DH * i:DH * (i + 1)],
                                             pts[i][:],
                                             start=(i == 0), stop=(i == nk - 1))
                            nc.tensor.matmul(ps_l[:], ones_sb[:, 0:1], pts[i][:],
                                             start=(i == 0), stop=(i == nk - 1))
                    lrec_f = ap2.tile([1, TBLK], F32, tag="lrec_f", name="lrec_f")
                    nc.vector.reciprocal_approx_fast(lrec_f[:], ps_l[:])
                    lrec = ap2.tile([1, TBLK], BF16, tag="lrec", name="lrec")
                    with nc.allow_low_precision(reason="1/l broadcast via bf16 matmul"):
                        nc.vector.tensor_copy(lrec[:], lrec_f[:])
                    ps_b = ps_tmp.tile([128, TBLK], F32, tag="tmp", name="bcast")
                    nc.tensor.matmul(ps_b[:], ones_sb[0:1, :], lrec[:],
                                     start=True, stop=True)
                    linv = ap2.tile([128, TBLK], F32, tag="linv", name="linv")
                    nc.scalar.activation(linv[:], ps_b[:], AF.Copy)
                    nc.vector.tensor_tensor(aT[h][:, qsl], ps_a[:], linv[:],
                                            op=ALU.mult)
                # head done: ship its token-owner slices to the exchange buf
                nc.gpsimd.dma_start(
                    a2a_in[h // 2][:].rearrange("(j r) t -> r j t", j=C)
                    [DH * (h % 2):DH * (h % 2 + 1), :, :],
                    aT[h][:].rearrange("d (j t) -> d j t", j=C))
                if h % 2 == 1:
                    nc.gpsimd.collective_compute(
                        "AllToAll", ALU.bypass, replica_groups=[list(range(C))],
                        ins=[a2a_in[h // 2][:].opt()],
                        outs=[a2a_out[h // 2][:].opt()])

        attn_ctx.close()

        # ================= P3: local wo over own tokens (token-major) ======
        # lhsT = a_own [hf, tok] (stationary), rhs = full Wo [hf, outfeat]
        # streamed as the moving operand -> psum [tok, outfeat]; evac fuses
        # the x residual add and the norm2 square partials.
        resid_ctx = ExitStack()
        resid = resid_ctx.enter_context(tc.tile_pool(name="resid", bufs=1))
        r2_sb = [resid.tile([128, D], F32, tag=f"r2_{t}", name=f"r2_{t}")
                 for t in range(TT)]
        sq_parts = {t: [] for t in range(TT)}
        with ExitStack() as ctx:
            wrhs = ctx.enter_context(tc.tile_pool(name="wo_w", bufs=2))
            wast = ctx.enter_context(tc.tile_pool(name="wo_a", bufs=1))
            wsq = ctx.enter_context(tc.tile_pool(name="wo_sq", bufs=2))
            # contraction order: head-pair 0 first so chains can start while
            # the pair-1 AllToAll is still in flight
            kp_order = ([4 * j + hh for hh in (0, 1) for j in range(C)] +
                        [4 * j + hh for hh in (2, 3) for j in range(C)])
            a_own = {}
            for g in kp_order:
                j, hh = g // 4, g % 4
                buf = a2a_out[hh // 2]
                r0 = 2 * DH * j + DH * (hh % 2)
                for t in range(TT):
                    at = wast.tile([128, 128], BF16, tag=f"a{g}_{t}",
                                   name=f"a{g}_{t}")
                    nc.scalar.dma_start(
                        at[:], buf[r0:r0 + DH, 128 * t:128 * (t + 1)])
                    a_own[(g, t)] = at
            for ob in range(D // TBLK):
                obs = slice(TBLK * ob, TBLK * (ob + 1))
                wts = {}
                for kp in kp_order:
                    wt = wrhs.tile([128, TBLK], BF16, tag=f"w{kp}",
                                   name=f"w{kp}")
                    nc.sync.dma_start(wt[:], woT.ap()[DH * kp:DH * (kp + 1), obs])
                    wts[kp] = wt
                for t in range(TT):
                    ps = ps_acc.tile([128, TBLK], F32, tag="acc", name="acc")
                    for ki, kp in enumerate(kp_order):
                        nc.tensor.matmul(
                            ps[:], a_own[(kp, t)][:], wts[kp][:],
                            start=(ki == 0), stop=(ki == len(kp_order) - 1))
                    xt_s = wsq.tile([128, TBLK], F32, tag="xs", name="xs")
                    nc.scalar.dma_start(
                        xt_s[:], x_c.ap()[128 * t:128 * (t + 1), obs])
                    nc.vector.tensor_tensor(r2_sb[t][:, obs], ps[:], xt_s[:],
                                            op=ALU.add)
                    sqs = wsq.tile([128, TBLK], F32, tag="sqs", name="sqs")
                    sp = resid.tile([128, 1], F32, tag=f"sqp{t}_{ob}",
                                    name=f"sqp{t}_{ob}")
                    nc.scalar.activation(sqs[:], r2_sb[t][:, obs], AF.Square,
                                         accum_out=sp[:])
                    sq_parts[t].append(sp)

        def transpose_add(src_chunks, cuts, pool, prefix, out_dram=None):
            """src_chunks: ReduceScatter outputs per chunk (feature-major).
            Transpose to token-major, add the r2 residual, store to out_dram."""
            for ch in range(len(cuts) - 1):
                W = cuts[ch + 1] - cuts[ch]
                for t in range(TT):
                    for gl in range(W * 128 // 512):
                        g = cuts[ch] * 128 // 512 + gl
                        gsl = slice(512 * g, 512 * (g + 1))
                        lt = pool.tile([128, 512], RSDT, tag=f"{prefix}lt",
                                       name=f"{prefix}lt")
                        nc.scalar.dma_start(
                            lt[:].rearrange("p (q t) -> p q t", q=4),
                            src_chunks[ch][512 * gl:512 * (gl + 1),
                                           128 * t:128 * (t + 1)]
                            .rearrange("(q d) t -> d q t", q=4))
                        ps = ps_tmp.tile([128, 512], BF16, tag="tmp", name="tps")
                        for q4 in range(4):
                            nc.tensor.transpose(
                                ps[:, 128 * q4:128 * (q4 + 1)],
                                lt[:, 128 * q4:128 * (q4 + 1)], identb_sb[:])
                        if True:
                            rsld = pool.tile([128, 512], F32, tag=f"{prefix}rs", name=f"{prefix}rs")
                            nc.scalar.dma_start(
                                rsld[:], r2d[128 * t:128 * (t + 1), gsl])
                            ot = pool.tile([128, 512], F32, tag=f"{prefix}ot", name=f"{prefix}ot")
                            nc.vector.tensor_tensor(ot[:], ps[:],
                                                    rsld[:], op=ALU.add)
                            nc.gpsimd.dma_start(
                                out_dram[128 * t:128 * (t + 1), gsl], ot[:])

        # ================= P4: norm2 + AllGather(h2) ============
        with ExitStack() as ctx:
            p4 = ctx.enter_context(tc.tile_pool(name="p4", bufs=1))
            seqpar_norm_and_gather(r2_sb, h2T_in_ch, h2T_all_ch, p4, ps_tmp,
                                   "n2", sq_partials=sq_parts)
            for t in range(TT):
                nc.gpsimd.dma_start(r2d[128 * t:128 * (t + 1), :], r2_sb[t][:])
        resid_ctx.close()

        # ================= P5: FFN =================
        h2_views = [h2T_all_ch[ch][:].rearrange("(b d) t -> d b t", b=C)
                    for ch in range(len(AG_CUTS) - 1)]
        fpart_views = [fpart_ch[ch][:].rearrange("(b d) t -> d b t", b=C)
                       for ch in range(len(RS2_CUTS) - 1)]
        with ExitStack() as ctx:
            fwp = ctx.enter_context(tc.tile_pool(name="ffn_w", bufs=2))
            fev = ctx.enter_context(tc.tile_pool(name="ffn_ev", bufs=3))
            fst = ctx.enter_context(tc.tile_pool(name="ffn_st", bufs=6))
            ftp = ctx.enter_context(tc.tile_pool(name="ffn_fT", bufs=1))
            frhs_ctx = ExitStack()
            frhs = frhs_ctx.enter_context(tc.tile_pool(name="ffn_rhs", bufs=1))
            fTs = [ftp.tile([128, N], BF16, tag=f"fT{m}", name=f"fT{m}")
                   for m in range(FM)]

            def load_gu_slabs(m):
                wg_s = fwp.tile([128, KP * 128], BF16, tag="wg", name="wg")
                nc.sync.dma_start(
                    wg_s[:].rearrange("p (k m) -> p k m", m=128),
                    wgT.ap()[:, 128 * m:128 * (m + 1)]
                    .rearrange("(k p) m -> p k m", p=128))
                wh_s = fwp.tile([128, KP * 128], BF16, tag="wh", name="wh")
                nc.sync.dma_start(
                    wh_s[:].rearrange("p (k m) -> p k m", m=128),
                    whT.ap()[:, 128 * m:128 * (m + 1)]
                    .rearrange("(k p) m -> p k m", p=128))
                return wg_s, wh_s

            # prefetch the first gate/up slabs BEFORE the AG2-gated rhs loads
            # so the gpsimd FIFO doesn't head-of-line block the prefetch
            next_slabs = load_gu_slabs(0)
            for hf in range(2):
                rts = []
                for kp in range(KP):
                    rt = frhs.tile([128, T2], BF16, tag=f"rhs{kp}", name=f"rhs{kp}")
                    chq, kpl = ch_of(AG_CUTS, kp)
                    nc.sync.dma_start(
                        rt[:].rearrange("p (b t) -> p b t", b=BPH),
                        h2_views[chq][128 * kpl:128 * (kpl + 1),
                                      BPH * hf:BPH * (hf + 1), :])
                    rts.append(rt)
                for m in range(FM):
                    wg_s, wh_s = next_slabs
                    nxt = (m + 1) if m + 1 < FM else (0 if hf == 0 else None)
                    if nxt is not None:
                        next_slabs = load_gu_slabs(nxt)
                    for ns in range(NS2):
                        ssl = slice(TBLK * ns, TBLK * (ns + 1))
                        osl = slice(T2 * hf + TBLK * ns,
                                    T2 * hf + TBLK * (ns + 1))
                        ps_g = ps_acc.tile([128, TBLK], F32, tag="acc", name="acc")
                        ps_u = ps_acc.tile([128, TBLK], F32, tag="acc", name="acc")
                        for kp in range(KP):
                            nc.tensor.matmul(
                                ps_g[:], wg_s[:, 128 * kp:128 * (kp + 1)],
                                rts[kp][:, ssl], start=(kp == 0),
                                stop=(kp == KP - 1))
                            nc.tensor.matmul(
                                ps_u[:], wh_s[:, 128 * kp:128 * (kp + 1)],
                                rts[kp][:, ssl], start=(kp == 0),
                                stop=(kp == KP - 1))
                        gs = fev.tile([128, TBLK], F32, tag="gs", name="gs")
                        if SILU_VIA_SIGMOID:
                            nc.scalar.activation(gs[:], ps_g[:], AF.Sigmoid)
                            gg = fev.tile([128, TBLK], F32, tag="gg", name="gg")
                            nc.vector.tensor_tensor(gg[:], ps_g[:], gs[:],
                                                    op=ALU.mult)
                            gs = gg
                        else:
                            nc.scalar.activation(gs[:], ps_g[:], AF.Silu)
                        nc.vector.tensor_tensor(fTs[m][:, osl], gs[:], ps_u[:],
                                                op=ALU.mult)
            frhs_ctx.close()
            # combined wf pass over all tokens; exchange chunks fire at
            # feature boundaries and overlap the rest of the pass
            for m2 in range(KP):
                wf_s = fwp.tile([128, FM * 128], BF16, tag="wf", name="wf")
                nc.sync.dma_start(
                    wf_s[:].rearrange("p (k m) -> p k m", m=128),
                    wfT.ap()[:, 128 * m2:128 * (m2 + 1)]
                    .rearrange("(k p) m -> p k m", p=128))
                ch2, m2l = ch_of(RS2_CUTS, m2)
                stage = fst.tile([128, N], RSDT, tag="st", name="st")
                for ns in range(N // TBLK):
                    ssl = slice(TBLK * ns, TBLK * (ns + 1))
                    ps = ps_acc.tile([128, TBLK], F32, tag="acc", name="acc")
                    for kp in range(FM):
                        nc.tensor.matmul(
                            ps[:], wf_s[:, 128 * kp:128 * (kp + 1)],
                            fTs[kp][:, ssl], start=(kp == 0),
                            stop=(kp == FM - 1))
                    if m2 % 2 == 0:
                        nc.scalar.activation(stage[:, ssl], ps[:], AF.Copy)
                    else:
                        nc.vector.tensor_copy(stage[:, ssl], ps[:])
                nc.scalar.dma_start(
                    fpart_views[ch2][128 * m2l:128 * (m2l + 1), :, :],
                    stage[:].rearrange("p (b t) -> p b t", b=C))
                if m2 == RS2_CUTS[ch2 + 1] - 1:
                    nc.gpsimd.collective_compute(
                        "ReduceScatter", ALU.add,
                        replica_groups=[list(range(C))],
                        ins=[fpart_ch[ch2][:].opt()],
                        outs=[fred_ch[ch2][:].opt()])

        # ================= P6: final residual add -> out =================
        with ExitStack() as ctx:
            p6 = ctx.enter_context(tc.tile_pool(name="p6", bufs=4))
            transpose_add(fred_ch, RS2_CUTS, p6, "p6", out_dram=out_c.ap())

    nc.compile()
    return nc


def make_in_maps(cfg, inputs):
    """Shard + transform the full fp32 inputs into per-core input maps."""
    N, D, QH, FC = cfg['N'], cfg['D'], cfg['QH'], cfg['FC']
    C = CORES
    NB = N // C
    bf = ml_dtypes.bfloat16
    f32 = np.float32

    x = np.ascontiguousarray(inputs['x'], dtype=f32)
    anw = np.asarray(inputs['attn_norm_w'], dtype=f32)
    fnw = np.asarray(inputs['ffn_norm_w'], dtype=f32)
    wq = np.asarray(inputs['wq'], dtype=f32) * anw[None, :]
    wk = np.asarray(inputs['wk'], dtype=f32) * anw[None, :]
    wv = np.asarray(inputs['wv'], dtype=f32) * anw[None, :]
    wo = np.asarray(inputs['wo'], dtype=f32)
    wg = np.asarray(inputs['wg'], dtype=f32) * fnw[None, :]
    wh = np.asarray(inputs['wh'], dtype=f32) * fnw[None, :]
    wf = np.asarray(inputs['wf'], dtype=f32)
    rcosT = np.ascontiguousarray(np.asarray(inputs['r_cos'], dtype=f32).T)
    rsinT = np.ascontiguousarray(np.asarray(inputs['r_sin'], dtype=f32).T)

    # rope swap as a matmul: swap(x) = P @ x ; lhsT = P.T
    P = np.zeros((DH, DH), dtype=f32)
    for i in range(DH // 2):
        P[2 * i, 2 * i + 1] = -1.0
        P[2 * i + 1, 2 * i] = 1.0
    swapT = np.ascontiguousarray(P.T)

    diagneg = np.diag(np.full(DH, NEG_BIG, dtype=f32))
    ident = np.eye(128, dtype=f32)
    ones = np.ones((128, 128), dtype=f32)
    m4 = np.zeros((4, 128, TBLK), dtype=f32)
    for ri in range(4):
        kk = np.arange(128)[:, None] + 128 * ri
        qq = np.arange(TBLK)[None, :]
        m4[ri] = (kk > qq).astype(f32)

    in_maps = []
    for c in range(C):
        qh_rows = slice(QH * DH * c, QH * DH * (c + 1))
        kv_rows = slice(DH * c, DH * (c + 1))
        fc_rows = slice(FC * c, FC * (c + 1))
        in_maps.append({
            "x_c": np.ascontiguousarray(x[NB * c:NB * (c + 1), :]),
            "wqT": np.ascontiguousarray(wq[qh_rows, :].T).astype(bf),
            "wkT": np.ascontiguousarray(wk[kv_rows, :].T).astype(bf),
            "wvT": np.ascontiguousarray(wv[kv_rows, :].T).astype(bf),
            "woT": np.ascontiguousarray(wo.T).astype(bf),
            "wgT": np.ascontiguousarray(wg[fc_rows, :].T).astype(bf),
            "whT": np.ascontiguousarray(wh[fc_rows, :].T).astype(bf),
            "wfT": np.ascontiguousarray(wf[:, fc_rows].T).astype(bf),
            "rcosT": rcosT,
            "rsinT": rsinT,
            "swapT": swapT.astype(bf),
            "diagneg": diagneg.astype(bf),
            "identb": ident.astype(bf),
            "onesc": ones.astype(bf),
            "masks": m4.astype(bf),
        })
    return in_maps


def assemble(results):
    return np.concatenate([r["out_c"] for r in results], axis=0)


_NC_CACHE = {}


def get_module(cfg_key=None):
    cfg = FULL_CFG if cfg_key is None else cfg_key
    key = tuple(sorted(cfg.items()))
    if key not in _NC_CACHE:
        _NC_CACHE[key] = build_module(cfg)
    return _NC_CACHE[key]


def run(inputs, cfg=None, trace=False):
    cfg = cfg or FULL_CFG
    nc = get_module(cfg)
    in_maps = make_in_maps(cfg, inputs)
    r = run_bass_kernel_spmd(nc, in_maps, list(range(CORES)), trace=trace)
    return assemble(r.results), r


def kernel(**inputs):
    out, _ = run(inputs)
    return np.asarray(out, dtype=np.float32)


# revision 20
# speedup vs baseline: 1.0780x; 1.0780x over previous
"""Tensor-parallel Llama layer on 8 Trainium2 NeuronCores (Bass/Tile).

Sharding: TP per the hint — wq/wk/wv/wg/wh column-sharded (4 q-heads + 1 kv
head + 1792 ffn rows per core), wo/wf row-sharded with ReduceScatter after
attention-out and ffn-out; sequence-parallel RMSNorms (256 tokens/core) with
AllGather of the normed activations (bf16).

Activations are kept feature-major (x.T layout) on chip so every projection
is a plain lhsT.T @ rhs with contraction on the partition axis. Weights are
pre-transposed and pre-cast to bf16 on the host (host prep is free).

v2 perf notes (from the v1 trace):
- PE runs GPIO-throttled at 13/16 of 2.4GHz; a dense N=512 matmul stream
  issues at ~263ns. LDWEIGHTS is already hidden; the wins are all in
  eliminating PE idle, not in restructuring matmuls.
- Collectives execute on ONE serial CC stream with a ~40us per-op floor:
  use few, large chunks (2 per collective), fire them as early as possible,
  and issue them from a dedicated queue (sync) since collective_compute
  blocks its issuing queue until completion.
- The PE MATMUL queue is strict FIFO: attention's score->exp->AV chain
  head-of-line blocks. Issue scores in waves of 3 so exp (scalar) pipelines
  behind the matmul stream.
- Prefetch loads on sync, collective-gated loads + partial-sum stores on
  scalar, collectives + residual/output stores on gpsimd.
"""
import sys

sys.path.insert(0, '/opt/trn_rl_repo')
from contextlib import ExitStack

import numpy as np
import ml_dtypes

import concourse.bass as bass
import concourse.tile as tile
from concourse import bacc, mybir
from concourse.bass_utils import run_bass_kernel_spmd

AF = mybir.ActivationFunctionType
ALU = mybir.AluOpType
BF16 = mybir.dt.bfloat16
F32 = mybir.dt.float32
F8E4 = mybir.dt.float8e4

CORES = 8
DH = 128
EPS = 1e-5
TBLK = 512
NEG_BIG = -1e30

FULL_CFG = dict(N=2048, D=4096, QH=4, FC=1792)

# CoreSim doesn't implement Silu; set True to build with Sigmoid + an extra
# multiply (same math) for simulator validation.
SILU_VIA_SIGMOID = False

# ReduceScatter the attention/ffn partial sums in bf16 (halves collective
# time); flip to False if accuracy needs the headroom.
RS_BF16 = True


def build_module(cfg):
    N, D, QH, FC = cfg['N'], cfg['D'], cfg['QH'], cfg['FC']
    C = CORES
    NB = N // C            # tokens per core block
    TT = NB // 128         # token tiles per core block
    KP = D // 128          # d_model contraction chunks
    NBLK = N // TBLK       # matmul token blocks
    BPT = TBLK // NB       # DRAM token-blocks per matmul token block
    KCH = N // DH          # attention k chunks
    QT = N // TBLK         # q tiles per head
    FM = FC // DH          # ffn M tiles per core
    T2 = N // 2            # ffn token half
    NS2 = T2 // TBLK       # 512-subblocks per ffn half
    BPH = C // 2           # DRAM token-blocks per ffn half
    MQKV = QH + 2
    scale = float(1.0 / np.sqrt(DH))

    nc = bacc.Bacc("TRN2", target_bir_lowering=False, debug=False, num_devices=C)

    x_c = nc.dram_tensor("x_c", [NB, D], F32, kind="ExternalInput")
    wqT = nc.dram_tensor("wqT", [D, QH * DH], BF16, kind="ExternalInput")
    wkT = nc.dram_tensor("wkT", [D, DH], BF16, kind="ExternalInput")
    wvT = nc.dram_tensor("wvT", [D, DH], BF16, kind="ExternalInput")
    woT = nc.dram_tensor("woT", [D // 1024, KP, 128, 1024], BF16,
                         kind="ExternalInput")
    wgT = nc.dram_tensor("wgT", [D, FC], BF16, kind="ExternalInput")
    whT = nc.dram_tensor("whT", [D, FC], BF16, kind="ExternalInput")
    wfT = nc.dram_tensor("wfT", [FC, D], BF16, kind="ExternalInput")
    rcosT = nc.dram_tensor("rcosT", [DH, N], F32, kind="ExternalInput")
    rsinT = nc.dram_tensor("rsinT", [DH, N], F32, kind="ExternalInput")
    swapT = nc.dram_tensor("swapT", [DH, DH], BF16, kind="ExternalInput")
    diagneg = nc.dram_tensor("diagneg", [DH, DH], BF16, kind="ExternalInput")
    identb = nc.dram_tensor("identb", [128, 128], BF16, kind="ExternalInput")
    onesc = nc.dram_tensor("onesc", [128, 128], BF16, kind="ExternalInput")
    masks = nc.dram_tensor("masks", [4, 128, TBLK], BF16, kind="ExternalInput")
    out_c = nc.dram_tensor("out_c", [NB, D], F32, kind="ExternalOutput")

    RSDT = BF16 if RS_BF16 else F32
    # Partial-sum exchanges are chunked ReduceScatters, each fired as soon
    # as its producer features are done (RS cost is ~linear in bytes, ~12us/MB
    # on this fabric; A2A and fp8 variants measured worse/too-inaccurate).
    AG_CUTS = [0, 32]                # norm-AllGathers (single shot: the
                                     # consumers need all features anyway)
    RS2_CUTS = [0, 8, 16, 24, 28, 32]  # ffn-out ReduceScatter (small tail)

    def ch_of(cuts, kp):
        for c in range(len(cuts) - 1):
            if kp < cuts[c + 1]:
                return c, kp - cuts[c]
        raise ValueError

    with tile.TileContext(nc) as tc, ExitStack() as top:
        dram = top.enter_context(tc.tile_pool(name="dram", bufs=1, space="DRAM"))

        def dram_chunks(nm, cuts, mul, dt, shared=False):
            kw = dict(addr_space="Shared") if shared else {}
            return [dram.tile([(cuts[i + 1] - cuts[i]) * 128 * mul, NB], dt,
                              tag=f"{nm}{i}", name=f"{nm}{i}", **kw)
                    for i in range(len(cuts) - 1)]

        r2d = dram.tile([NB, D], F32, tag="r2d", name="r2d")
        hT_in_ch = dram_chunks("hT_in", AG_CUTS, 1, BF16)
        hT_all_ch = dram_chunks("hT_all", AG_CUTS, C, BF16, shared=True)
        # attention-out exchange: AllToAll re-shards a from head-sharded to
        # token-sharded (2.1MB per core vs a 16.8MB ReduceScatter); wo then
        # runs locally per core over its own tokens with full Wo streamed.
        a2a_in = [dram.tile([C * 2 * DH, NB], BF16, tag=f"a2ai{p}",
                            name=f"a2ai{p}") for p in range(2)]
        a2a_out = [dram.tile([C * 2 * DH, NB], BF16, tag=f"a2ao{p}",
                             name=f"a2ao{p}") for p in range(2)]
        h2T_in_ch = dram_chunks("h2T_in", AG_CUTS, 1, BF16)
        h2T_all_ch = dram_chunks("h2T_all", AG_CUTS, C, BF16, shared=True)
        fpart_ch = dram_chunks("fpart", RS2_CUTS, C, RSDT)
        fred_ch = dram_chunks("fred", RS2_CUTS, 1, RSDT)

        # ---- constants resident in SBUF (loads on gpsimd queue) ----
        const = top.enter_context(tc.tile_pool(name="const", bufs=1))
        identb_sb = const.tile([128, 128], BF16, tag="identb", name="identb")
        swap_sb = const.tile([DH, DH], BF16, tag="swap", name="swap")
        diag_sb = const.tile([DH, DH], BF16, tag="diag", name="diag")
        ones_sb = const.tile([128, 128], BF16, tag="ones", name="ones")
        masks_sb = const.tile([128, 4 * TBLK], BF16, tag="masks", name="masks")

        # ---- shared PSUM pools (4+3+1 = 8 banks) ----
        ps_acc = top.enter_context(tc.tile_pool(name="ps_acc", bufs=4, space="PSUM"))
        ps_tmp = top.enter_context(tc.tile_pool(name="ps_tmp", bufs=3, space="PSUM"))
        ps_sml = top.enter_context(tc.tile_pool(name="ps_sml", bufs=1, space="PSUM"))

        # ---- attention residents (freed after P3; opened last for LIFO) ----
        attn_ctx = ExitStack()
        attn = attn_ctx.enter_context(tc.tile_pool(name="attn", bufs=1))
        rcos_sb = attn.tile([DH, N], F32, tag="rcos", name="rcos")
        rsin_sb = attn.tile([DH, N], F32, tag="rsin", name="rsin")
        qrot = [attn.tile([DH, N], BF16, tag=f"qrot{h}", name=f"qrot{h}") for h in range(QH)]
        krot = attn.tile([DH, N], BF16, tag="krot", name="krot")
        vsb = attn.tile([DH, N], BF16, tag="vsb", name="vsb")
        vtok = attn.tile([128, KCH * DH], BF16, tag="vtok", name="vtok")
        aT = [attn.tile([DH, N], BF16, tag=f"aT{h}", name=f"aT{h}") for h in range(QH)]

        def seqpar_norm_and_gather(src_tiles, dst_chunks, out_chunks, pool,
                                   pspool, prefix, sq_partials=None):
            """src_tiles: TT SBUF tiles [128, D] f32 (token-major rows of this
            core's block). Phase A: RMS-normalize each row (scalar+vector).
            Phase B: chunk-ordered transpose to feature-major, store, and fire
            the AllGather for each chunk as soon as its stores are queued."""
            htoks = []
            for t in range(TT):
                xt = src_tiles[t]
                if sq_partials is None:
                    sq = pool.tile([128, D], F32, tag=f"{prefix}sq", name=f"{prefix}sq")
                    ssum = pool.tile([128, 1], F32, tag=f"{prefix}ss", name=f"{prefix}ss")
                    nc.scalar.activation(sq[:], xt[:], AF.Square, accum_out=ssum[:])
                else:
                    acc = sq_partials[t][0]
                    for pi, p in enumerate(sq_partials[t][1:]):
                        ns = pool.tile([128, 1], F32, tag=f"{prefix}ss{pi % 2}",
                                       name=f"{prefix}ss")
                        nc.vector.tensor_tensor(ns[:], acc[:], p[:], op=ALU.add)
                        acc = ns
                    ssum = acc
                var = pool.tile([128, 1], F32, tag=f"{prefix}var", name=f"{prefix}var")
                nc.vector.tensor_scalar(
                    out=var[:], in0=ssum[:], scalar1=1.0 / D, scalar2=EPS,
                    op0=ALU.mult, op1=ALU.add)
                sv = pool.tile([128, 1], F32, tag=f"{prefix}sv", name=f"{prefix}sv")
                nc.scalar.activation(sv[:], var[:], AF.Sqrt)
                rstd = pool.tile([128, 1], F32, tag=f"{prefix}rstd", name=f"{prefix}rstd")
                nc.vector.reciprocal(rstd[:], sv[:])
                htok = pool.tile([128, D], BF16, tag=f"{prefix}h{t}", name=f"{prefix}h{t}")
                nc.vector.tensor_scalar_mul(htok[:], xt[:], rstd[:])
                htoks.append(htok)
            for ch in range(len(AG_CUTS) - 1):
                for t in range(TT):
                    for gl in range((AG_CUTS[ch + 1] - AG_CUTS[ch]) // 4):
                        g = AG_CUTS[ch] // 4 + gl
                        ps = pspool.tile([128, 512], BF16, tag="tmp", name="tps")
                        for q4 in range(4):
                            dd = 4 * g + q4
                            nc.tensor.transpose(
                                ps[:, 128 * q4:128 * (q4 + 1)],
                                htoks[t][:, 128 * dd:128 * (dd + 1)], identb_sb[:])
                        ev = pool.tile([128, 512], BF16, tag=f"{prefix}ev{gl % 4}", name=f"{prefix}ev")
                        if g % 2 == 0:
                            nc.vector.tensor_copy(ev[:], ps[:])
                        else:
                            nc.scalar.activation(ev[:], ps[:], AF.Copy)
                        r0 = 128 * 4 * gl
                        nc.gpsimd.dma_start(
                            dst_chunks[ch][r0:r0 + 512, 128 * t:128 * (t + 1)]
                            .rearrange("(q d) t -> d q t", q=4),
                            ev[:].rearrange("p (q t) -> p q t", q=4))
                nc.gpsimd.collective_compute(
                    "AllGather", ALU.bypass, replica_groups=[list(range(C))],
                    ins=[dst_chunks[ch][:].opt()], outs=[out_chunks[ch][:].opt()])

        # ================= P0: norm1 (seq-parallel) + chunked AllGather ====
        p0_ctx = ExitStack()
        p0 = p0_ctx.enter_context(tc.tile_pool(name="p0", bufs=1))
        x_tiles = []
        for t in range(TT):
            xt = p0.tile([128, D], F32, tag=f"x{t}", name=f"x{t}")
            nc.sync.dma_start(xt[:], x_c.ap()[128 * t:128 * (t + 1), :])
            x_tiles.append(xt)
        # consts after x on the same load queue
        nc.sync.dma_start(identb_sb[:], identb.ap())
        nc.sync.dma_start(swap_sb[:], swapT.ap())
        nc.sync.dma_start(diag_sb[:], diagneg.ap())
        nc.sync.dma_start(ones_sb[:], onesc.ap())
        nc.sync.dma_start(
            masks_sb[:].rearrange("p (r t) -> p r t", r=4),
            masks.ap().rearrange("r p t -> p r t"))
        seqpar_norm_and_gather(x_tiles, hT_in_ch, hT_all_ch, p0, ps_tmp, "n1")
        p0_ctx.close()

        hT_views = [hT_all_ch[ch][:].rearrange("(b d) t -> d b t", b=C)
                    for ch in range(len(AG_CUTS) - 1)]

        # ================= P1: QKV + RoPE (per token block) =================
        with ExitStack() as ctx:
            wsl = ctx.enter_context(tc.tile_pool(name="qkv_w", bufs=1))
            rhsp = ctx.enter_context(tc.tile_pool(name="qkv_rhs", bufs=2))
            ep = ctx.enter_context(tc.tile_pool(name="qkv_ep", bufs=3))
            # QKV weight slabs are small (6 x 8KB/partition bf16): load once
            slabs = []
            for m in range(MQKV):
                slab = wsl.tile([128, KP * 128], BF16, tag=f"w{m}", name=f"w{m}")
                if m < QH:
                    src = wqT.ap()[:, 128 * m:128 * (m + 1)]
                elif m == QH:
                    src = wkT.ap()
                else:
                    src = wvT.ap()
                nc.sync.dma_start(
                    slab[:].rearrange("p (k m) -> p k m", m=128),
                    src.rearrange("(k p) m -> p k m", p=128))
                slabs.append(slab)
            # rope tables after slabs (needed later than slab m=0)
            nc.sync.dma_start(rcos_sb[:], rcosT.ap())
            nc.sync.dma_start(rsin_sb[:], rsinT.ap())

            def rope(dst, src_sb, ps_swap, sl):
                """dst[:, sl] = src*cos + (P@src)*sin; src_sb bf16, ps_swap psum."""
                t1 = ep.tile([128, TBLK], F32, tag="rope_t1", name="rope_t1")
                nc.vector.tensor_tensor(t1[:], src_sb[:], rcos_sb[:, sl], op=ALU.mult)
                t2 = ep.tile([128, TBLK], F32, tag="rope_t2", name="rope_t2")
                nc.vector.tensor_tensor(t2[:], ps_swap[:], rsin_sb[:, sl], op=ALU.mult)
                nc.vector.tensor_tensor(dst[:, sl], t1[:], t2[:], op=ALU.add)

            for nb in range(NBLK):
                sl = slice(TBLK * nb, TBLK * (nb + 1))
                # one rhs load per (nb, kp), shared by both M-groups
                rtiles = []
                for kp in range(KP):
                    rt = rhsp.tile([128, TBLK], BF16, tag=f"rhs{kp}", name=f"rhs{kp}")
                    chq, kpl = ch_of(AG_CUTS, kp)
                    nc.sync.dma_start(
                        rt[:].rearrange("p (b t) -> p b t", b=BPT),
                        hT_views[chq][128 * kpl:128 * (kpl + 1),
                                      BPT * nb:BPT * (nb + 1), :])
                    rtiles.append(rt)
                # emit both m-groups' matmul chains first (dense PE stream),
                # then the rope/evac work, so the PE queue never head-of-line
                # blocks on scalar evacs.
                pending = []
                for hm in range(2):
                    group = list(range(3 * hm, min(3 * (hm + 1), MQKV)))
                    gacc = {m: ps_acc.tile([128, TBLK], F32, tag="acc", name="acc") for m in group}
                    for kp in range(KP):
                        for m in group:
                            nc.tensor.matmul(
                                gacc[m][:], slabs[m][:, 128 * kp:128 * (kp + 1)],
                                rtiles[kp][:], start=(kp == 0), stop=(kp == KP - 1))
                    # evacs (scalar) can run while the next group's chains run
                    for m in group:
                        ps = gacc[m]
                        if m <= QH:  # q heads and k need rope
                            sb = ep.tile([128, TBLK], BF16, tag=f"qk_sb{m}", name="qk_sb")
                            nc.scalar.activation(sb[:], ps[:], AF.Copy)
                            pending.append((m, sb))
                        else:  # v: plain copy
                            nc.scalar.activation(vsb[:, sl], ps[:], AF.Copy)
                # rope swaps (PE) + vector rope, after all chains queued
                for m, sb in pending:
                    ps_swap = ps_tmp.tile([128, TBLK], F32, tag="tmp", name="swp")
                    nc.tensor.matmul(ps_swap[:], swap_sb[:], sb[:],
                                     start=True, stop=True)
                    dst = qrot[m] if m < QH else krot
                    rope(dst, sb, ps_swap, sl)
                # transpose this block's v chunks to token-major
                for q4 in range(BPT * NB // 128):
                    i = (TBLK * nb) // 128 + q4
                    psv = ps_tmp.tile([128, 512], BF16, tag="tmp", name="vtp")
                    nc.tensor.transpose(
                        psv[:, 128 * (i % 4):128 * (i % 4) + 128],
                        vsb[:, 128 * i:128 * (i + 1)], identb_sb[:])
                    nc.vector.tensor_copy(
                        vtok[:, 128 * i:128 * (i + 1)],
                        psv[:, 128 * (i % 4):128 * (i % 4) + 128])

        # ================= P2: attention =================
        # Per (head, q-tile): k-chunks processed in waves of 3 so the PE
        # matmul stream (score/AV/lsum) never head-of-line blocks on the
        # scalar exp.
        with ExitStack() as ctx:
            pp = ctx.enter_context(tc.tile_pool(name="att_p", bufs=6))
            ap2 = ctx.enter_context(tc.tile_pool(name="att_t", bufs=4))
            for h in range(QH):
                for j in range(QT):
                    qsl = slice(TBLK * j, TBLK * (j + 1))
                    nk = (TBLK * (j + 1)) // DH
                    ps_a = ps_acc.tile([128, TBLK], F32, tag="acc", name="acc")
                    ps_l = ps_sml.tile([1, TBLK], F32, tag="lsum", name="lsum")
                    kpj = TBLK // DH  # k chunks per q tile (straddle count)
                    for i0 in range(0, nk, 3):
                        wave = range(i0, min(i0 + 3, nk))
                        pts = {}
                        for i in wave:
                            ps_s = ps_tmp.tile([128, TBLK], F32, tag="tmp", name="score")
                            diagonal = i >= kpj * j
                            nc.tensor.matmul(
                                ps_s[:], krot[:, DH * i:DH * (i + 1)], qrot[h][:, qsl],
                                start=True, stop=not diagonal)
                            if diagonal:
                                ri = i - kpj * j
                                nc.tensor.matmul(
                                    ps_s[:], diag_sb[:],
                                    masks_sb[:, TBLK * ri:TBLK * (ri + 1)],
                                    start=False, stop=True)
                            pt = pp.tile([128, TBLK], BF16, tag="p", name="p")
                            nc.scalar.activation(pt[:], ps_s[:], AF.Exp, scale=scale)
                            pts[i] = pt
                        for i in wave:
                            nc.tensor.matmul(ps_a[:], vtok[:, DH * i:DH * (i + 1)],
                                             pts[i][:],
                                             start=(i == 0), stop=(i == nk - 1))
                            nc.tensor.matmul(ps_l[:], ones_sb[:, 0:1], pts[i][:],
                                             start=(i == 0), stop=(i == nk - 1))
                    lrec_f = ap2.tile([1, TBLK], F32, tag="lrec_f", name="lrec_f")
                    nc.vector.reciprocal_approx_fast(lrec_f[:], ps_l[:])
                    lrec = ap2.tile([1, TBLK], BF16, tag="lrec", name="lrec")
                    with nc.allow_low_precision(reason="1/l broadcast via bf16 matmul"):
                        nc.vector.tensor_copy(lrec[:], lrec_f[:])
                    ps_b = ps_tmp.tile([128, TBLK], F32, tag="tmp", name="bcast")
                    nc.tensor.matmul(ps_b[:], ones_sb[0:1, :], lrec[:],
                                     start=True, stop=True)
                    linv = ap2.tile([128, TBLK], F32, tag="linv", name="linv")
                    nc.scalar.activation(linv[:], ps_b[:], AF.Copy)
                    nc.vector.tensor_tensor(aT[h][:, qsl], ps_a[:], linv[:],
                                            op=ALU.mult)
                # head done: ship its token-owner slices to the exchange buf
                nc.gpsimd.dma_start(
                    a2a_in[h // 2][:].rearrange("(j r) t -> r j t", j=C)
                    [DH * (h % 2):DH * (h % 2 + 1), :, :],
                    aT[h][:].rearrange("d (j t) -> d j t", j=C))
                if h % 2 == 1:
                    nc.gpsimd.collective_compute(
                        "AllToAll", ALU.bypass, replica_groups=[list(range(C))],
                        ins=[a2a_in[h // 2][:].opt()],
                        outs=[a2a_out[h // 2][:].opt()])

        attn_ctx.close()

        # ================= P3: local wo over own tokens (token-major) ======
        # lhsT = a_own [hf, tok] (stationary), rhs = full Wo [hf, outfeat]
        # streamed as the moving operand -> psum [tok, outfeat]; evac fuses
        # the x residual add and the norm2 square partials.
        resid_ctx = ExitStack()
        resid = resid_ctx.enter_context(tc.tile_pool(name="resid", bufs=1))
        r2_sb = [resid.tile([128, D], F32, tag=f"r2_{t}", name=f"r2_{t}")
                 for t in range(TT)]
        sq_parts = {t: [] for t in range(TT)}
        with ExitStack() as ctx:
            wrhs = ctx.enter_context(tc.tile_pool(name="wo_w", bufs=2))
            wast = ctx.enter_context(tc.tile_pool(name="wo_a", bufs=1))
            wsq = ctx.enter_context(tc.tile_pool(name="wo_sq", bufs=2))
            # contraction order: head-pair 0 first so chains can start while
            # the pair-1 AllToAll is still in flight
            kp_order = ([4 * j + hh for hh in (0, 1) for j in range(C)] +
                        [4 * j + hh for hh in (2, 3) for j in range(C)])
            a_own = {}
            for g in kp_order:
                j, hh = g // 4, g % 4
                buf = a2a_out[hh // 2]
                r0 = 2 * DH * j + DH * (hh % 2)
                for t in range(TT):
                    at = wast.tile([128, 128], BF16, tag=f"a{g}_{t}",
                                   name=f"a{g}_{t}")
                    nc.scalar.dma_start(
                        at[:], buf[r0:r0 + DH, 128 * t:128 * (t + 1)])
                    a_own[(g, t)] = at
            for ob2 in range(D // 1024):
                wts = {}
                for wi, kp in enumerate(kp_order):
                    wt = wrhs.tile([128, 1024], BF16, tag=f"w{kp}",
                                   name=f"w{kp}")
                    eng = nc.sync if wi % 2 == 0 else nc.gpsimd
                    eng.dma_start(wt[:], woT.ap()[ob2, kp, :, :])
                    wts[kp] = wt
                for half in range(2):
                    ob = 2 * ob2 + half
                    obs = slice(TBLK * ob, TBLK * (ob + 1))
                    hsl = slice(TBLK * half, TBLK * (half + 1))
                    for t in range(TT):
                        ps = ps_acc.tile([128, TBLK], F32, tag="acc", name="acc")
                        for ki, kp in enumerate(kp_order):
                            nc.tensor.matmul(
                                ps[:], a_own[(kp, t)][:], wts[kp][:, hsl],
                                start=(ki == 0), stop=(ki == len(kp_order) - 1))
                        xt_s = wsq.tile([128, TBLK], F32, tag="xs", name="xs")
                        nc.scalar.dma_start(
                            xt_s[:], x_c.ap()[128 * t:128 * (t + 1), obs])
                        nc.vector.tensor_tensor(r2_sb[t][:, obs], ps[:], xt_s[:],
                                                op=ALU.add)
                        sqs = wsq.tile([128, TBLK], F32, tag="sqs", name="sqs")
                        sp = resid.tile([128, 1], F32, tag=f"sqp{t}_{ob}",
                                        name=f"sqp{t}_{ob}")
                        nc.scalar.activation(sqs[:], r2_sb[t][:, obs], AF.Square,
                                             accum_out=sp[:])
                        sq_parts[t].append(sp)

        def transpose_add(src_chunks, cuts, pool, prefix, out_dram=None):
            """src_chunks: ReduceScatter outputs per chunk (feature-major).
            Transpose to token-major, add the r2 residual, store to out_dram."""
            for ch in range(len(cuts) - 1):
                W = cuts[ch + 1] - cuts[ch]
                for t in range(TT):
                    for gl in range(W * 128 // 512):
                        g = cuts[ch] * 128 // 512 + gl
                        gsl = slice(512 * g, 512 * (g + 1))
                        lt = pool.tile([128, 512], RSDT, tag=f"{prefix}lt",
                                       name=f"{prefix}lt")
                        nc.scalar.dma_start(
                            lt[:].rearrange("p (q t) -> p q t", q=4),
                            src_chunks[ch][512 * gl:512 * (gl + 1),
                                           128 * t:128 * (t + 1)]
                            .rearrange("(q d) t -> d q t", q=4))
                        ps = ps_tmp.tile([128, 512], BF16, tag="tmp", name="tps")
                        for q4 in range(4):
                            nc.tensor.transpose(
                                ps[:, 128 * q4:128 * (q4 + 1)],
                                lt[:, 128 * q4:128 * (q4 + 1)], identb_sb[:])
                        if True:
                            rsld = pool.tile([128, 512], F32, tag=f"{prefix}rs", name=f"{prefix}rs")
                            nc.scalar.dma_start(
                                rsld[:], r2d[128 * t:128 * (t + 1), gsl])
                            ot = pool.tile([128, 512], F32, tag=f"{prefix}ot", name=f"{prefix}ot")
                            nc.vector.tensor_tensor(ot[:], ps[:],
                                                    rsld[:], op=ALU.add)
                            nc.gpsimd.dma_start(
                                out_dram[128 * t:128 * (t + 1), gsl], ot[:])

        # ================= P4: norm2 + AllGather(h2) ============
        with ExitStack() as ctx:
            p4 = ctx.enter_context(tc.tile_pool(name="p4", bufs=1))
            seqpar_norm_and_gather(r2_sb, h2T_in_ch, h2T_all_ch, p4, ps_tmp,
                                   "n2", sq_partials=sq_parts)
            for t in range(TT):
                nc.gpsimd.dma_start(r2d[128 * t:128 * (t + 1), :], r2_sb[t][:])
        resid_ctx.close()

        # ================= P5: FFN =================
        h2_views = [h2T_all_ch[ch][:].rearrange("(b d) t -> d b t", b=C)
                    for ch in range(len(AG_CUTS) - 1)]
        fpart_views = [fpart_ch[ch][:].rearrange("(b d) t -> d b t", b=C)
                       for ch in range(len(RS2_CUTS) - 1)]
        with ExitStack() as ctx:
            fwp = ctx.enter_context(tc.tile_pool(name="ffn_w", bufs=2))
            fev = ctx.enter_context(tc.tile_pool(name="ffn_ev", bufs=3))
            fst = ctx.enter_context(tc.tile_pool(name="ffn_st", bufs=6))
            ftp = ctx.enter_context(tc.tile_pool(name="ffn_fT", bufs=1))
            frhs_ctx = ExitStack()
            frhs = frhs_ctx.enter_context(tc.tile_pool(name="ffn_rhs", bufs=1))
            fTs = [ftp.tile([128, N], BF16, tag=f"fT{m}", name=f"fT{m}")
                   for m in range(FM)]

            def load_gu_slabs(m):
                wg_s = fwp.tile([128, KP * 128], BF16, tag="wg", name="wg")
                nc.sync.dma_start(
                    wg_s[:].rearrange("p (k m) -> p k m", m=128),
                    wgT.ap()[:, 128 * m:128 * (m + 1)]
                    .rearrange("(k p) m -> p k m", p=128))
                wh_s = fwp.tile([128, KP * 128], BF16, tag="wh", name="wh")
                nc.sync.dma_start(
                    wh_s[:].rearrange("p (k m) -> p k m", m=128),
                    whT.ap()[:, 128 * m:128 * (m + 1)]
                    .rearrange("(k p) m -> p k m", p=128))
                return wg_s, wh_s

            # prefetch the first gate/up slabs BEFORE the AG2-gated rhs loads
            # so the gpsimd FIFO doesn't head-of-line block the prefetch
            next_slabs = load_gu_slabs(0)
            for hf in range(2):
                rts = []
                for kp in range(KP):
                    rt = frhs.tile([128, T2], BF16, tag=f"rhs{kp}", name=f"rhs{kp}")
                    chq, kpl = ch_of(AG_CUTS, kp)
                    nc.sync.dma_start(
                        rt[:].rearrange("p (b t) -> p b t", b=BPH),
                        h2_views[chq][128 * kpl:128 * (kpl + 1),
                                      BPH * hf:BPH * (hf + 1), :])
                    rts.append(rt)
                for m in range(FM):
                    wg_s, wh_s = next_slabs
                    nxt = (m + 1) if m + 1 < FM else (0 if hf == 0 else None)
                    if nxt is not None:
                        next_slabs = load_gu_slabs(nxt)
                    for ns in range(NS2):
                        ssl = slice(TBLK * ns, TBLK * (ns + 1))
                        osl = slice(T2 * hf + TBLK * ns,
                                    T2 * hf + TBLK * (ns + 1))
                        ps_g = ps_acc.tile([128, TBLK], F32, tag="acc", name="acc")
                        ps_u = ps_acc.tile([128, TBLK], F32, tag="acc", name="acc")
                        for kp in range(KP):
                            nc.tensor.matmul(
                                ps_g[:], wg_s[:, 128 * kp:128 * (kp + 1)],
                                rts[kp][:, ssl], start=(kp == 0),
                                stop=(kp == KP - 1))
                            nc.tensor.matmul(
                                ps_u[:], wh_s[:, 128 * kp:128 * (kp + 1)],
                                rts[kp][:, ssl], start=(kp == 0),
                                stop=(kp == KP - 1))
                        gs = fev.tile([128, TBLK], F32, tag="gs", name="gs")
                        if SILU_VIA_SIGMOID:
                            nc.scalar.activation(gs[:], ps_g[:], AF.Sigmoid)
                            gg = fev.tile([128, TBLK], F32, tag="gg", name="gg")
                            nc.vector.tensor_tensor(gg[:], ps_g[:], gs[:],
                                                    op=ALU.mult)
                            gs = gg
                        else:
                            nc.scalar.activation(gs[:], ps_g[:], AF.Silu)
                        nc.vector.tensor_tensor(fTs[m][:, osl], gs[:], ps_u[:],
                                                op=ALU.mult)
            frhs_ctx.close()
            # combined wf pass over all tokens; exchange chunks fire at
            # feature boundaries and overlap the rest of the pass
            for m2 in range(KP):
                wf_s = fwp.tile([128, FM * 128], BF16, tag="wf", name="wf")
                nc.sync.dma_start(
                    wf_s[:].rearrange("p (k m) -> p k m", m=128),
                    wfT.ap()[:, 128 * m2:128 * (m2 + 1)]
                    .rearrange("(k p) m -> p k m", p=128))
                ch2, m2l = ch_of(RS2_CUTS, m2)
                stage = fst.tile([128, N], RSDT, tag="st", name="st")
                for ns in range(N // TBLK):
                    ssl = slice(TBLK * ns, TBLK * (ns + 1))
                    ps = ps_acc.tile([128, TBLK], F32, tag="acc", name="acc")
                    for kp in range(FM):
                        nc.tensor.matmul(
                            ps[:], wf_s[:, 128 * kp:128 * (kp + 1)],
                            fTs[kp][:, ssl], start=(kp == 0),
                            stop=(kp == FM - 1))
                    if m2 % 2 == 0:
                        nc.scalar.activation(stage[:, ssl], ps[:], AF.Copy)
                    else:
                        nc.vector.tensor_copy(stage[:, ssl], ps[:])
                nc.scalar.dma_start(
                    fpart_views[ch2][128 * m2l:128 * (m2l + 1), :, :],
                    stage[:].rearrange("p (b t) -> p b t", b=C))
                if m2 == RS2_CUTS[ch2 + 1] - 1:
                    nc.gpsimd.collective_compute(
                        "ReduceScatter", ALU.add,
                        replica_groups=[list(range(C))],
                        ins=[fpart_ch[ch2][:].opt()],
                        outs=[fred_ch[ch2][:].opt()])

        # ================= P6: final residual add -> out =================
        with ExitStack() as ctx:
            p6 = ctx.enter_context(tc.tile_pool(name="p6", bufs=4))
            transpose_add(fred_ch, RS2_CUTS, p6, "p6", out_dram=out_c.ap())

    nc.compile()
    return nc


def make_in_maps(cfg, inputs):
    """Shard + transform the full fp32 inputs into per-core input maps."""
    N, D, QH, FC = cfg['N'], cfg['D'], cfg['QH'], cfg['FC']
    C = CORES
    NB = N // C
    bf = ml_dtypes.bfloat16
    f32 = np.float32

    KP0 = D // 128
    x = np.ascontiguousarray(inputs['x'], dtype=f32)
    anw = np.asarray(inputs['attn_norm_w'], dtype=f32)
    fnw = np.asarray(inputs['ffn_norm_w'], dtype=f32)
    wq = np.asarray(inputs['wq'], dtype=f32) * anw[None, :]
    wk = np.asarray(inputs['wk'], dtype=f32) * anw[None, :]
    wv = np.asarray(inputs['wv'], dtype=f32) * anw[None, :]
    wo = np.asarray(inputs['wo'], dtype=f32)
    wg = np.asarray(inputs['wg'], dtype=f32) * fnw[None, :]
    wh = np.asarray(inputs['wh'], dtype=f32) * fnw[None, :]
    wf = np.asarray(inputs['wf'], dtype=f32)
    rcosT = np.ascontiguousarray(np.asarray(inputs['r_cos'], dtype=f32).T)
    rsinT = np.ascontiguousarray(np.asarray(inputs['r_sin'], dtype=f32).T)

    # rope swap as a matmul: swap(x) = P @ x ; lhsT = P.T
    P = np.zeros((DH, DH), dtype=f32)
    for i in range(DH // 2):
        P[2 * i, 2 * i + 1] = -1.0
        P[2 * i + 1, 2 * i] = 1.0
    swapT = np.ascontiguousarray(P.T)

    diagneg = np.diag(np.full(DH, NEG_BIG, dtype=f32))
    ident = np.eye(128, dtype=f32)
    ones = np.ones((128, 128), dtype=f32)
    m4 = np.zeros((4, 128, TBLK), dtype=f32)
    for ri in range(4):
        kk = np.arange(128)[:, None] + 128 * ri
        qq = np.arange(TBLK)[None, :]
        m4[ri] = (kk > qq).astype(f32)

    in_maps = []
    for c in range(C):
        qh_rows = slice(QH * DH * c, QH * DH * (c + 1))
        kv_rows = slice(DH * c, DH * (c + 1))
        fc_rows = slice(FC * c, FC * (c + 1))
        in_maps.append({
            "x_c": np.ascontiguousarray(x[NB * c:NB * (c + 1), :]),
            "wqT": np.ascontiguousarray(wq[qh_rows, :].T).astype(bf),
            "wkT": np.ascontiguousarray(wk[kv_rows, :].T).astype(bf),
            "wvT": np.ascontiguousarray(wv[kv_rows, :].T).astype(bf),
            "woT": np.ascontiguousarray(
                wo.T.reshape(KP0, 128, 4, 1024).transpose(2, 0, 1, 3)).astype(bf),
            "wgT": np.ascontiguousarray(wg[fc_rows, :].T).astype(bf),
            "whT": np.ascontiguousarray(wh[fc_rows, :].T).astype(bf),
            "wfT": np.ascontiguousarray(wf[:, fc_rows].T).astype(bf),
            "rcosT": rcosT,
            "rsinT": rsinT,
            "swapT": swapT.astype(bf),
            "diagneg": diagneg.astype(bf),
            "identb": ident.astype(bf),
            "onesc": ones.astype(bf),
            "masks": m4.astype(bf),
        })
    return in_maps


def assemble(results):
    return np.concatenate([r["out_c"] for r in results], axis=0)


_NC_CACHE = {}


def get_module(cfg_key=None):
    cfg = FULL_CFG if cfg_key is None else cfg_key
    key = tuple(sorted(cfg.items()))
    if key not in _NC_CACHE:
        _NC_CACHE[key] = build_module(cfg)
    return _NC_CACHE[key]


def run(inputs, cfg=None, trace=False):
    cfg = cfg or FULL_CFG
    nc = get_module(cfg)
    in_maps = make_in_maps(cfg, inputs)
    r = run_bass_kernel_spmd(nc, in_maps, list(range(CORES)), trace=trace)
    return assemble(r.results), r


def kernel(**inputs):
    out, _ = run(inputs)
    return np.asarray(out, dtype=np.float32)
